# revision 7
# baseline (speedup 1.0000x reference)
"""Trainium2 Bass kernel for a 3-layer MLP forward pass.

Network: 784 -> 128 (relu) -> 64 (relu) -> 10 (linear), biases folded as the
last row of each weight matrix. Batch 65536, pure data parallel over 8 cores
(8192 rows each).

Strategy:
  * Host pre-transposes each X shard to feature-major layout so the kernel
    streams fully contiguous DMAs and never transposes on-chip.
  * Weights are stationary matmul operands (W is already [K_in, M_out]); all
    weights+biases ride in ONE packed const DMA so downstream instructions
    only ever wait on a single DMA completion semaphore (the Matmult/LDWEIGHTS
    wait table only fits one wait).
  * All three PSUM drains run on the ScalarEngine (relu+bias fused via the
    activation bias port; the final linear uses Identity+bias), keeping PE's
    cross-engine waits limited to single ACT ticks.
  * Output is produced feature-major [10, 8192] per core; host transposes back.
"""

import numpy as np

import concourse.bass as bass
import concourse.mybir as mybir
from concourse import bacc
from concourse.bass_utils import run_bass_kernel_spmd
from concourse.tile import TileContext

N_CORES = 8
BATCH = 65536
B_SHARD = BATCH // N_CORES  # 8192
F_IN = 784
H1, H2, H3 = 128, 64, 10
KC = 112  # contraction chunk (784 = 7 * 112)
NCHUNK = 7
BT = 512  # batch tile = matmul moving free dim (fp32 max 512)
NT = B_SHARD // BT

# packed const layout (columns of a [128, CW] tile)
W0_OFF = 0  # [112, 7*128] but stored in 128-row chunks of width 128
W1_OFF = NCHUNK * H1  # 896: [128, 64]
W2_OFF = W1_OFF + H2  # 960: [64, 10]
B0_OFF = W2_OFF + H3  # 970: [128, 1]
B1_OFF = B0_OFF + 1  # 971: [64, 1]
B2_OFF = B1_OFF + 1  # 972: [10, 1]
CW = B2_OFF + 1  # 973

F32 = mybir.dt.float32
RELU = mybir.ActivationFunctionType.Relu
IDENT = mybir.ActivationFunctionType.Identity


def _build_bass() -> bass.Bass:
    nc = bacc.Bacc()

    x = nc.dram_tensor("x", [NT, KC, NCHUNK * BT], F32, kind="ExternalInput")
    consts = nc.dram_tensor("consts", [128, CW], F32, kind="ExternalInput")
    y = nc.dram_tensor("y", [H3, B_SHARD], F32, kind="ExternalOutput")

    with TileContext(nc) as tc:
        with (
            tc.tile_pool(name="cp", bufs=1) as cp,
            tc.tile_pool(name="xp", bufs=4) as xp,
            tc.tile_pool(name="hp", bufs=3) as hp,
            tc.tile_pool(name="yp", bufs=1) as yp,
            tc.tile_pool(name="scratchp", bufs=1) as scratchp,
            tc.tile_pool(name="ps0p", bufs=1, space="PSUM") as ps0p,
            tc.tile_pool(name="ps1p", bufs=2, space="PSUM") as ps1p,
            tc.tile_pool(name="ps2p", bufs=2, space="PSUM") as ps2p,
            tc.tile_pool(name="ps3p", bufs=2, space="PSUM") as ps3p,
        ):
            cons = cp.tile([128, CW], F32)
            nc.sync.dma_start(cons, consts[:])

            b0t = cons[0:H1, B0_OFF : B0_OFF + 1]
            b1t = cons[0:H2, B1_OFF : B1_OFF + 1]
            b2t = cons[0:H3, B2_OFF : B2_OFF + 1]

            ybuf = yp.tile([H3, B_SHARD], F32)

            # Warm-up: absorb the const-DMA wait on PE and ACT so no later
            # matmul ever needs two semaphore waits.
            ps0 = ps0p.tile([128, BT], F32)
            nc.tensor.matmul(
                ps0,
                lhsT=cons[0:KC, 0:128],
                rhs=cons[0:KC, 0:BT],
                start=True,
                stop=True,
            )
            actwarm = scratchp.tile([128, 1], F32)
            nc.scalar.activation(actwarm, cons[:, B0_OFF : B0_OFF + 1], RELU)

            for t in range(NT):
                xt = xp.tile([KC, NCHUNK * BT], F32, tag="xt")
                nc.sync.dma_start(xt, x[t])

                ps1 = ps1p.tile([H1, BT], F32, tag="ps1")
                for c in range(NCHUNK):
                    nc.tensor.matmul(
                        ps1,
                        lhsT=cons[0:KC, c * H1 : (c + 1) * H1],
                        rhs=xt[:, c * BT : (c + 1) * BT],
                        start=(c == 0),
                        stop=(c == NCHUNK - 1),
                    )
                h1 = hp.tile([H1, BT], F32, tag="h1")
                nc.scalar.activation(h1, ps1, RELU, bias=b0t)

                ps2 = ps2p.tile([H2, BT], F32, tag="ps2")
                nc.tensor.matmul(
                    ps2,
                    lhsT=cons[0:H1, W1_OFF : W1_OFF + H2],
                    rhs=h1,
                    start=True,
                    stop=True,
                )
                h2 = hp.tile([H2, BT], F32, tag="h2")
                nc.scalar.activation(h2, ps2, RELU, bias=b1t)

                ps3 = ps3p.tile([H3, BT], F32, tag="ps3")
                nc.tensor.matmul(
                    ps3,
                    lhsT=cons[0:H2, W2_OFF : W2_OFF + H3],
                    rhs=h2,
                    start=True,
                    stop=True,
                )
                nc.scalar.activation(
                    ybuf[:, t * BT : (t + 1) * BT], ps3, IDENT, bias=b2t
                )

            nc.sync.dma_start(y[:], ybuf)

    nc.finalize()
    return nc


_CACHED_NC: bass.Bass | None = None


def _get_nc() -> bass.Bass:
    global _CACHED_NC
    if _CACHED_NC is None:
        _CACHED_NC = _build_bass()
    return _CACHED_NC


def _pack_consts(W0: np.ndarray, W1: np.ndarray, W2: np.ndarray) -> np.ndarray:
    cons = np.zeros((128, CW), dtype=np.float32)
    w0r = W0[:F_IN].reshape(NCHUNK, KC, H1).transpose(1, 0, 2).reshape(KC, NCHUNK * H1)
    cons[0:KC, W0_OFF : W0_OFF + NCHUNK * H1] = w0r
    cons[0:H1, W1_OFF : W1_OFF + H2] = W1[:H1]
    cons[0:H2, W2_OFF : W2_OFF + H3] = W2[:H2]
    cons[0:H1, B0_OFF] = W0[F_IN]
    cons[0:H2, B1_OFF] = W1[H1]
    cons[0:H3, B2_OFF] = W2[H2]
    return cons


def _prep_shard(x_shard: np.ndarray) -> np.ndarray:
    """[8192, 784] batch-major -> [NT, KC, NCHUNK*BT] feature-major tiles."""
    xs = x_shard.reshape(NT, BT, NCHUNK, KC).transpose(0, 3, 2, 1)
    return np.ascontiguousarray(xs).reshape(NT, KC, NCHUNK * BT)


def kernel(X: np.ndarray, W0: np.ndarray, W1: np.ndarray, W2: np.ndarray, **_kw):
    X = np.ascontiguousarray(X, dtype=np.float32)
    cons = _pack_consts(
        np.asarray(W0, dtype=np.float32),
        np.asarray(W1, dtype=np.float32),
        np.asarray(W2, dtype=np.float32),
    )

    in_maps = []
    for c in range(N_CORES):
        shard = X[c * B_SHARD : (c + 1) * B_SHARD]
        in_maps.append({"x": _prep_shard(shard), "consts": cons})

    res = run_bass_kernel_spmd(_get_nc(), in_maps, core_ids=list(range(N_CORES)))
    global LAST_RESULT
    LAST_RESULT = res
    out = np.concatenate([r["y"].T for r in res.results], axis=0)
    return np.ascontiguousarray(out)


LAST_RESULT = None


if __name__ == "__main__":
    rng = np.random.default_rng(0)
    X = rng.standard_normal((BATCH, F_IN), dtype=np.float32)
    W0 = rng.random((F_IN + 1, H1), dtype=np.float32) * 0.1
    W1 = rng.random((H1 + 1, H2), dtype=np.float32) * 0.1
    W2 = rng.random((H2 + 1, H3), dtype=np.float32) * 0.1
    y = kernel(X=X, W0=W0, W1=W1, W2=W2)
    print(y.shape, y.dtype, y[:2])


# revision 8
# speedup vs baseline: 1.3875x; 1.3875x over previous
"""Trainium2 Bass kernel for a 3-layer MLP forward pass.

Network: 784 -> 128 (relu) -> 64 (relu) -> 10 (linear), biases folded as the
last row of each weight matrix. Batch 65536, pure data parallel over 8 cores
(8192 rows each).

Strategy:
  * Host pre-transposes each X shard to feature-major layout so the kernel
    streams fully contiguous DMAs and never transposes on-chip.
  * Weights are stationary matmul operands (W is already [K_in, M_out]); all
    weights+biases ride in ONE packed const DMA so downstream instructions
    only ever wait on a single DMA completion semaphore (the Matmult/LDWEIGHTS
    wait table only fits one wait).
  * All three PSUM drains run on the ScalarEngine (relu+bias fused via the
    activation bias port; the final linear uses Identity+bias), keeping PE's
    cross-engine waits limited to single ACT ticks.
  * Output is produced feature-major [10, 8192] per core; host transposes back.
"""

import numpy as np

import concourse.bass as bass
import concourse.mybir as mybir
from concourse import bacc
from concourse.bass_utils import run_bass_kernel_spmd
from concourse.tile import TileContext

N_CORES = 8
BATCH = 65536
B_SHARD = BATCH // N_CORES  # 8192
F_IN = 784
H1, H2, H3 = 128, 64, 10
KC = 112  # contraction chunk (784 = 7 * 112)
NCHUNK = 7
BT = 512  # batch tile = matmul moving free dim (fp32 max 512)
NT = B_SHARD // BT

# packed const layout (columns of a [128, CW] tile)
W0_OFF = 0  # [112, 7*128] but stored in 128-row chunks of width 128
W1_OFF = NCHUNK * H1  # 896: [128, 64]
W2_OFF = W1_OFF + H2  # 960: [64, 10]
B0_OFF = W2_OFF + H3  # 970: [128, 1]
B1_OFF = B0_OFF + 1  # 971: [64, 1]
B2_OFF = B1_OFF + 1  # 972: [10, 1]
CW = B2_OFF + 1  # 973

F32 = mybir.dt.float32
F32R = mybir.dt.float32r
RELU = mybir.ActivationFunctionType.Relu
IDENT = mybir.ActivationFunctionType.Identity


def _build_bass() -> bass.Bass:
    nc = bacc.Bacc()

    x = nc.dram_tensor("x", [NT, KC, NCHUNK * BT], F32R, kind="ExternalInput")
    consts = nc.dram_tensor("consts", [128, CW], F32R, kind="ExternalInput")
    y = nc.dram_tensor("y", [H3, B_SHARD], F32, kind="ExternalOutput")

    with TileContext(nc) as tc:
        with (
            tc.tile_pool(name="cp", bufs=1) as cp,
            tc.tile_pool(name="xp", bufs=4) as xp,
            tc.tile_pool(name="hp", bufs=3) as hp,
            tc.tile_pool(name="yp", bufs=1) as yp,
            tc.tile_pool(name="scratchp", bufs=1) as scratchp,
            tc.tile_pool(name="ps0p", bufs=1, space="PSUM") as ps0p,
            tc.tile_pool(name="ps1p", bufs=2, space="PSUM") as ps1p,
            tc.tile_pool(name="ps2p", bufs=2, space="PSUM") as ps2p,
            tc.tile_pool(name="ps3p", bufs=2, space="PSUM") as ps3p,
        ):
            cons = cp.tile([128, CW], F32R)
            nc.sync.dma_start(cons, consts[:])

            b0t = cons[0:H1, B0_OFF : B0_OFF + 1]
            b1t = cons[0:H2, B1_OFF : B1_OFF + 1]
            b2t = cons[0:H3, B2_OFF : B2_OFF + 1]

            ybuf = yp.tile([H3, B_SHARD], F32)

            # Warm-up: absorb the const-DMA wait on PE and ACT so no later
            # matmul ever needs two semaphore waits.
            ps0 = ps0p.tile([128, BT], F32)
            nc.tensor.matmul(
                ps0,
                lhsT=cons[0:KC, 0:128],
                rhs=cons[0:KC, 0:BT],
                start=True,
                stop=True,
            )
            actwarm = scratchp.tile([128, 1], F32R)
            nc.scalar.activation(actwarm, cons[:, B0_OFF : B0_OFF + 1], RELU)

            for t in range(NT):
                xt = xp.tile([KC, NCHUNK * BT], F32R, tag="xt")
                nc.sync.dma_start(xt, x[t])

                ps1 = ps1p.tile([H1, BT], F32, tag="ps1")
                for c in range(NCHUNK):
                    nc.tensor.matmul(
                        ps1,
                        lhsT=cons[0:KC, c * H1 : (c + 1) * H1],
                        rhs=xt[:, c * BT : (c + 1) * BT],
                        start=(c == 0),
                        stop=(c == NCHUNK - 1),
                    )
                h1 = hp.tile([H1, BT], F32R, tag="h1")
                nc.scalar.activation(h1, ps1, RELU, bias=b0t)

                ps2 = ps2p.tile([H2, BT], F32, tag="ps2")
                nc.tensor.matmul(
                    ps2,
                    lhsT=cons[0:H1, W1_OFF : W1_OFF + H2],
                    rhs=h1,
                    start=True,
                    stop=True,
                )
                h2 = hp.tile([H2, BT], F32R, tag="h2")
                nc.scalar.activation(h2, ps2, RELU, bias=b1t)

                ps3 = ps3p.tile([H3, BT], F32, tag="ps3")
                nc.tensor.matmul(
                    ps3,
                    lhsT=cons[0:H2, W2_OFF : W2_OFF + H3],
                    rhs=h2,
                    start=True,
                    stop=True,
                )
                nc.scalar.activation(
                    ybuf[:, t * BT : (t + 1) * BT], ps3, IDENT, bias=b2t
                )

            nc.sync.dma_start(y[:], ybuf)

    nc.finalize()
    return nc


_CACHED_NC: bass.Bass | None = None


def _get_nc() -> bass.Bass:
    global _CACHED_NC
    if _CACHED_NC is None:
        _CACHED_NC = _build_bass()
    return _CACHED_NC


def _pack_consts(W0: np.ndarray, W1: np.ndarray, W2: np.ndarray) -> np.ndarray:
    cons = np.zeros((128, CW), dtype=np.float32)
    w0r = W0[:F_IN].reshape(NCHUNK, KC, H1).transpose(1, 0, 2).reshape(KC, NCHUNK * H1)
    cons[0:KC, W0_OFF : W0_OFF + NCHUNK * H1] = w0r
    cons[0:H1, W1_OFF : W1_OFF + H2] = W1[:H1]
    cons[0:H2, W2_OFF : W2_OFF + H3] = W2[:H2]
    cons[0:H1, B0_OFF] = W0[F_IN]
    cons[0:H2, B1_OFF] = W1[H1]
    cons[0:H3, B2_OFF] = W2[H2]
    return cons


def _prep_shard(x_shard: np.ndarray) -> np.ndarray:
    """[8192, 784] batch-major -> [NT, KC, NCHUNK*BT] feature-major tiles."""
    xs = x_shard.reshape(NT, BT, NCHUNK, KC).transpose(0, 3, 2, 1)
    return np.ascontiguousarray(xs).reshape(NT, KC, NCHUNK * BT)


def kernel(X: np.ndarray, W0: np.ndarray, W1: np.ndarray, W2: np.ndarray, **_kw):
    X = np.ascontiguousarray(X, dtype=np.float32)
    cons = _pack_consts(
        np.asarray(W0, dtype=np.float32),
        np.asarray(W1, dtype=np.float32),
        np.asarray(W2, dtype=np.float32),
    )

    in_maps = []
    for c in range(N_CORES):
        shard = X[c * B_SHARD : (c + 1) * B_SHARD]
        in_maps.append({"x": _prep_shard(shard), "consts": cons})

    res = run_bass_kernel_spmd(_get_nc(), in_maps, core_ids=list(range(N_CORES)))
    global LAST_RESULT
    LAST_RESULT = res
    out = np.concatenate([r["y"].T for r in res.results], axis=0)
    return np.ascontiguousarray(out)


LAST_RESULT = None


if __name__ == "__main__":
    rng = np.random.default_rng(0)
    X = rng.standard_normal((BATCH, F_IN), dtype=np.float32)
    W0 = rng.random((F_IN + 1, H1), dtype=np.float32) * 0.1
    W1 = rng.random((H1 + 1, H2), dtype=np.float32) * 0.1
    W2 = rng.random((H2 + 1, H3), dtype=np.float32) * 0.1
    y = kernel(X=X, W0=W0, W1=W1, W2=W2)
    print(y.shape, y.dtype, y[:2])
